# revision 4
# baseline (speedup 1.0000x reference)
"""Multi-head self-attention (B=4, N=2048, C=768, H=12, D=64) on 8 TRN2 NeuronCores.

Sharding: (batch, head-group) - core c handles batch c//2, heads (c%2)*6..+5.
Each core computes its 6 heads' attention plus the partial output projection;
the host sums the two partials per batch and adds the bias terms.

v2 dataflow (ACT-exp is the roofline: 192 exps x ~1147ns = 220us; everything
else is organized to keep ScalarE saturated from ~14us to the end):
  - warm-up matmul burst at t=0 keeps the PE HAM un-throttled through the
    input-DMA window (otherwise phase 1 runs at 1.2 GHz until ~31us)
  - merged per-pair Q^T/K^T tiles: head 2t on partitions 0-63, head 2t+1 on
    64-127 (no zero padding). mm2 runs as two K=64 matmuls in different PE
    row-groups (tile_position (0,0)/(64,0)) writing one [128,1024] psum tile,
    so both heads' S^T tiles stream concurrently through the array.
  - one exp (N=1024) per (pair, q-chunk, k-tile) covers both heads.
  - mm3 accumulates per head into 1-bank psum ([128,512] per q-chunk);
    denominator via ones-columns in V_aug as in v1.
  - psum banks: psP 2x2 + psoA/psoB 1+1 + psF 2 = 8.
  - phase-1 (V_aug, later pairs' Q^T/K^T) and proj tiles 0..11 ride as
    fillers in the per-window PE slack; only proj(12..15) remains at the tail.
Host: out[b] = part[2b] + part[2b+1] + (qkv_b_v @ proj_w + proj_b)
"""

import numpy as np
import ml_dtypes

B, N, C = 4, 2048, 768
H, D = 12, 64
SCALE = D ** -0.5
HL = 6            # heads per core
QK = HL * D       # 384, width of q (= k = v) section per core
KS = 6            # contraction subtiles (768 = 6*128 rows)
P = 128
NT = N
SC = 512          # q-chunk width
NCH = NT // SC    # 4 q-chunks
MT = N // P       # 16 k-token tiles

_cache = {}


def _build():
    import concourse.bass as bass
    import concourse.mybir as mybir
    import concourse.tile as tile
    from concourse import bacc

    f32 = mybir.dt.float32
    bf16 = mybir.dt.bfloat16

    nc = bacc.Bacc(None, target_bir_lowering=False)
    xt_d = nc.declare_dram_parameter("xt", [KS * P, NT], bf16, isOutput=False)
    wq_d = nc.declare_dram_parameter("wq", [KS * P, 3 * QK], bf16, isOutput=False)
    wp_d = nc.declare_dram_parameter("wp", [QK, C], bf16, isOutput=False)
    bias_d = nc.declare_dram_parameter("bias_qk", [P, 2 * QK // P], f32, isOutput=False)
    out_d = nc.declare_dram_parameter("out", [NT, C], f32, isOutput=True)

    xt_r = xt_d.rearrange("(o p) n -> p o n", p=P)
    wq_r = wq_d.rearrange("(o p) n -> p o n", p=P)
    wp_r = wp_d.rearrange("(o p) n -> p o n", p=P)

    with tile.TileContext(nc) as tc:
        with (
            tc.tile_pool(name="persist", bufs=1) as persist,
            tc.tile_pool(name="e_pool", bufs=3) as e_pool,
            tc.tile_pool(name="un_pool", bufs=4) as un_pool,
            tc.tile_pool(name="rec_pool", bufs=4) as rec_pool,
            tc.tile_pool(name="bc_pool", bufs=4) as bc_pool,
            tc.tile_pool(name="stage_pool", bufs=3) as stage_pool,
            tc.tile_pool(name="dr", bufs=4, space="DRAM") as dr_pool,
            tc.tile_pool(name="psP", bufs=2, space="PSUM") as psP,
            tc.tile_pool(name="psO", bufs=1, space="PSUM") as psO,
            tc.tile_pool(name="psF", bufs=2, space="PSUM") as psF,
        ):
            xt = persist.tile([P, KS, NT], bf16)
            wq = persist.tile([P, KS, 3 * QK], bf16)
            wp = persist.tile([P, QK // P, C], bf16)
            # merged per-pair Q^T/K^T: head 2t dims on partitions 0-63,
            # head 2t+1 dims on partitions 64-127
            qp = [persist.tile([P, NT], bf16, name=f"qp{t}") for t in range(3)]
            kp = [persist.tile([P, NT], bf16, name=f"kp{t}") for t in range(3)]
            vv = persist.tile([P, MT, HL, P], bf16)     # V_aug per token-tile/head
            outt = [persist.tile([P, NT], bf16, name=f"outt{o}")
                    for o in range(QK // P)]            # normalized out^T
            bias_qk = persist.tile([P, 2 * QK // P], f32)
            warm = persist.tile([P, SC], bf16)

            eng = [nc.sync, nc.scalar, nc.gpsimd]

            # PE warm-up: back-to-back matmuls with no DMA dependency keep the
            # HAM activity window busy through the input-load phase so real
            # phase-1 work runs at 2.4 GHz from the start.
            nc.vector.memset(warm[:, :], 0.0)
            ps_w = psF.tile([P, SC], f32, tag="fps", name="ps_w")
            NWARM = 44
            for i in range(NWARM):
                nc.tensor.matmul(
                    ps_w[:, :SC], lhsT=warm[:, 0:P], rhs=warm[:, :SC],
                    start=(i == 0), stop=(i == NWARM - 1),
                )

            # input DMA: critical set (bias, wq-q, wq-k, xt) round-robins all
            # three queues; wq-v and wp follow behind.
            nc.sync.dma_start(bias_qk[:, :], bias_d[:, :])
            qi = [0]

            def dma(dst, src):
                eng[qi[0] % 3].dma_start(dst, src)
                qi[0] += 1

            for o in range(KS):
                dma(wq[:, o, 0:QK], wq_r[:, o, 0:QK])
                dma(wq[:, o, QK:2 * QK], wq_r[:, o, QK:2 * QK])
            for j in range(NCH):
                for o in range(KS):
                    dma(xt[:, o, j * SC:(j + 1) * SC], xt_r[:, o, j * SC:(j + 1) * SC])
            for o in range(KS):
                dma(wq[:, o, 2 * QK:3 * QK], wq_r[:, o, 2 * QK:3 * QK])
            for o in range(QK // P):
                dma(wp[:, o, :], wp_r[:, o, :])

            def qkt_chunk(mi, j, half=None):
                # one 512-col chunk of [Q^T;K^T] rows mi*128.. (mi<3 -> Q).
                # half=0: matmuls o 0..2 (returns psum tile), half=1: o 3..5
                # + bias add. half=None: both.
                dst = qp if mi < 3 else kp
                t = mi % 3
                o_lo = 3 if half == 1 else 0
                o_hi = 3 if half == 0 else KS
                if half == 1:
                    ps = _qkt_ps.pop((mi, j))
                else:
                    ps = psF.tile([P, SC], f32, tag="fps", name="ps_f")
                for o in range(o_lo, o_hi):
                    nc.tensor.matmul(
                        ps[:, :SC],
                        lhsT=wq[:, o, mi * P:(mi + 1) * P],
                        rhs=xt[:, o, j * SC:(j + 1) * SC],
                        start=(o == 0),
                        stop=(o == KS - 1),
                    )
                if half == 0:
                    _qkt_ps[(mi, j)] = ps
                    return
                sc = slice(j * SC, (j + 1) * SC)
                nc.vector.tensor_scalar_add(
                    out=dst[t][0:64, sc], in0=ps[0:64, :SC],
                    scalar1=bias_qk[0:64, mi:mi + 1])
                nc.vector.tensor_scalar_add(
                    out=dst[t][64:P, sc], in0=ps[64:P, :SC],
                    scalar1=bias_qk[64:P, mi:mi + 1])

            _qkt_ps = {}

            def v_mtile(ti):
                ps = psF.tile([P, SC], f32, tag="fps", name="ps_f")
                for o in range(KS):
                    nc.tensor.matmul(
                        ps[:, :QK],
                        lhsT=xt[:, o, ti * P:(ti + 1) * P],
                        rhs=wq[:, o, 2 * QK:3 * QK],
                        start=(o == 0),
                        stop=(o == KS - 1),
                    )
                psv = ps[:, :QK].rearrange("p (h d) -> p h d", h=HL)
                nc.vector.tensor_copy(out=vv[:, ti, 0:HL:2, 0:64], in_=psv[:, 0:HL:2, :])
                nc.vector.tensor_copy(out=vv[:, ti, 1:HL:2, 64:128], in_=psv[:, 1:HL:2, :])

            def proj_part(ti, pi):
                # half pi of the output projection for token tile ti
                w0, wn = (0, 512) if pi == 0 else (512, 256)
                if pi == 0:
                    stage = stage_pool.tile([P, C], f32)
                    _stage[ti] = stage
                else:
                    stage = _stage.pop(ti)
                ps = psF.tile([P, SC], f32, tag="fps", name="ps_pj")
                for o in range(QK // P):
                    nc.tensor.matmul(
                        ps[:, :wn],
                        lhsT=outt[o][:, ti * P:(ti + 1) * P],
                        rhs=wp[:, o, w0:w0 + wn],
                        start=(o == 0),
                        stop=(o == QK // P - 1),
                    )
                nc.vector.tensor_copy(out=stage[:, w0:w0 + wn], in_=ps[:, :wn])
                if pi == 1:
                    eng[ti % 3].dma_start(out_d[ti * P:(ti + 1) * P, :], stage[:, :])

            _stage = {}

            # filler queues: independent PE work injected one per window
            urgent = []
            lazy = []

            def run_filler():
                if urgent:
                    urgent.pop(0)()
                elif lazy:
                    lazy.pop(0)()

            def normalize(t, qc, po, h, nck=1):
                # drain psum, then reciprocal-of-denominator broadcast and the
                # normalized write into outt (off the critical path)
                dlane = 64 if h % 2 == 0 else 32
                po_lo = (h % 2) * 64
                un = un_pool.tile([P, SC], f32, tag="un", name="un")
                nc.vector.tensor_copy(out=un[:, :], in_=po[:, :])
                cw = SC // nck
                for ck in range(nck):
                    lo = ck * cw
                    cs = slice(qc * SC + lo, qc * SC + lo + cw)
                    dn = dr_pool.tile([1, cw], f32, name="dn", tag="dn")
                    eng[(h + qc) % 3].dma_start(dn[:, :], un[dlane:dlane + 1, lo:lo + cw])
                    dnp = rec_pool.tile([P, cw // P], f32, name="dnp", tag="dnp")
                    eng[(h + qc + 1) % 3].dma_start(dnp[:, :], dn[0].rearrange("(p f) -> p f", p=P))
                    rcp = rec_pool.tile([P, cw // P], f32, name="rcp", tag="rcp")
                    nc.vector.reciprocal(rcp[:, :], dnp[:, :])
                    rd = dr_pool.tile([1, cw], f32, name="rd", tag="rd")
                    eng[(h + qc + 2) % 3].dma_start(rd[0].rearrange("(p f) -> p f", p=P), rcp[:, :])
                    bc = bc_pool.tile([P, cw], f32, name="bc", tag="bc")
                    eng[(h + qc) % 3].dma_start(
                        bc[:, :],
                        bass.AP(tensor=rd.tensor, offset=rd.offset, ap=[[0, P]] + list(rd.ap)),
                    )
                    nc.vector.tensor_mul(
                        outt[t][po_lo:po_lo + 64, cs], un[po_lo:po_lo + 64, lo:lo + cw],
                        bc[po_lo:po_lo + 64, :],
                    )

            def pair_chunk(t, qc, nck=1):
                # 16 windows: paired mm2 (row-groups 0/1) -> exp -> per-head mm3
                po_a = psO.tile([P, SC], f32, tag="poA", name="po_a")
                po_b = psO.tile([P, SC], f32, tag="poB", name="po_b")
                qs = slice(qc * SC, (qc + 1) * SC)
                for m in range(MT):
                    ps = psP.tile([P, 2 * SC], f32, tag="ps", name="ps_s")
                    mt = slice(m * P, (m + 1) * P)
                    nc.tensor.matmul(
                        ps[:, 0:SC], lhsT=kp[t][0:64, mt], rhs=qp[t][0:64, qs],
                        start=True, stop=True,
                    )
                    nc.tensor.matmul(
                        ps[:, SC:2 * SC], lhsT=kp[t][64:P, mt], rhs=qp[t][64:P, qs],
                        start=True, stop=True,
                    )
                    e = e_pool.tile([P, 2 * SC], bf16)
                    nc.scalar.activation(
                        e[:, :], ps[:, :], mybir.ActivationFunctionType.Exp,
                        scale=float(SCALE),
                    )
                    run_filler()
                    nc.tensor.matmul(
                        po_a[:, :SC], lhsT=vv[:, m, 2 * t, :], rhs=e[:, 0:SC],
                        start=(m == 0), stop=(m == MT - 1),
                    )
                    nc.tensor.matmul(
                        po_b[:, :SC], lhsT=vv[:, m, 2 * t + 1, :], rhs=e[:, SC:2 * SC],
                        start=(m == 0), stop=(m == MT - 1),
                    )
                normalize(t, qc, po_a, 2 * t, nck=nck)
                normalize(t, qc, po_b, 2 * t + 1, nck=nck)

            # prologue: pair-0 Q^T/K^T and the first V tiles
            for j in range(NCH):
                qkt_chunk(0, j)
            for j in range(NCH):
                qkt_chunk(3, j)
            # V_aug col layout (denominator via ones) - issued after the
            # prologue bias-adds so the DVE FIFO serves those first:
            #   even heads: [v(64) | ones(32) | zeros(32)]  -> den at row 64
            #   odd  heads: [zeros(32) | ones(32) | v(64)]  -> den at row 32
            for h in range(HL):
                nc.vector.memset(vv[:, :, h, 32:96] if h % 2 else vv[:, :, h, 64:96], 1.0)
                nc.vector.memset(vv[:, :, h, 0:32] if h % 2 else vv[:, :, h, 96:128], 0.0)
            for ti in range(4):
                v_mtile(ti)

            # fillers for pair 0's windows: remaining V tiles (urgent: one per
            # window, 4-tile lookahead covers mm3's consumption)
            for ti in range(4, MT):
                urgent.append(lambda ti=ti: v_mtile(ti))
            # pair 1 inputs, split in half-chunks to fit window slack
            for mi in (1, 4):
                for j in range(NCH):
                    lazy.append(lambda mi=mi, j=j: qkt_chunk(mi, j, half=0))
                    lazy.append(lambda mi=mi, j=j: qkt_chunk(mi, j, half=1))

            for qc in range(NCH):
                pair_chunk(0, qc)
            for mi in (2, 5):
                for j in range(NCH):
                    lazy.append(lambda mi=mi, j=j: qkt_chunk(mi, j, half=0))
                    lazy.append(lambda mi=mi, j=j: qkt_chunk(mi, j, half=1))
            for qc in range(NCH):
                pair_chunk(1, qc)
            for qc in range(NCH):
                # after pair-2 chunk qc-1 is normalized, its token tiles can
                # be projected; ride them as fillers in the later chunks
                if qc > 0:
                    for ti in range(4 * (qc - 1), 4 * qc):
                        lazy.append(lambda ti=ti: proj_part(ti, 0))
                        lazy.append(lambda ti=ti: proj_part(ti, 1))
                pair_chunk(2, qc, nck=2 if qc == NCH - 1 else 1)

            while urgent or lazy:
                (urgent if urgent else lazy).pop(0)()
            for ti in range(12, MT):
                proj_part(ti, 0)
                proj_part(ti, 1)

    nc.compile()
    return nc


def _prep_inputs(x, qkv_w, qkv_b):
    bf = ml_dtypes.bfloat16
    in_maps = []
    for c in range(8):
        b, hs = c // 2, (c % 2) * HL
        xt = np.ascontiguousarray(x[b].T.astype(bf))
        wq = np.zeros((KS * P, 3 * QK), dtype=bf)
        for s in range(3):  # q, k, v sections
            cols = qkv_w[:, s * C + hs * D: s * C + (hs + HL) * D]
            wq[0:C, s * QK:(s + 1) * QK] = cols.astype(bf)
        qk_bias = np.concatenate([
            qkv_b[hs * D:(hs + HL) * D], qkv_b[C + hs * D: C + (hs + HL) * D]
        ]).astype(np.float32)
        in_maps.append({"xt": xt, "wq": wq,
                        "bias_qk": np.ascontiguousarray(qk_bias.reshape(6, P).T)})
    return in_maps


def kernel(x, qkv_w, qkv_b, proj_w, proj_b):
    from concourse.bass_utils import run_bass_kernel_spmd

    x = np.asarray(x, dtype=np.float32)
    qkv_w = np.asarray(qkv_w, dtype=np.float32)
    qkv_b = np.asarray(qkv_b, dtype=np.float32)
    proj_w = np.asarray(proj_w, dtype=np.float32)
    proj_b = np.asarray(proj_b, dtype=np.float32)

    if "nc" not in _cache:
        _cache["nc"] = _build()
    nc = _cache["nc"]

    bf = ml_dtypes.bfloat16
    in_maps = _prep_inputs(x, qkv_w, qkv_b)
    for c in range(8):
        hs = (c % 2) * HL
        in_maps[c]["wp"] = proj_w[hs * D:(hs + HL) * D, :].astype(bf)

    res = run_bass_kernel_spmd(nc, in_maps, core_ids=list(range(8)))
    parts = [res.results[c]["out"].astype(np.float32) for c in range(8)]

    # v-bias contribution (exact, f32) + proj bias, added once per batch
    const_row = qkv_b[2 * C:] @ proj_w + proj_b
    out = np.empty((B, N, C), dtype=np.float32)
    for b in range(B):
        out[b] = parts[2 * b] + parts[2 * b + 1] + const_row
    return out
